# revision 1
# baseline (speedup 1.0000x reference)
"""Trainium2 Bass kernel for nn_CruxMiniCircuit (gnn_message_passing).

Reference semantics: B independent rows; each row is a circuit of N nodes
(literal nodes hold a fixed one-hot distribution over 10 ints, op nodes
combine left/right child distributions through a per-op bilinear table
followed by softmax).  The reference runs 10 synchronous passes over all
nodes and returns only the root (node 0) logits per row.

Key observation: the output depends only on node 0's dependency cone
unrolled 10 passes deep.  Literal children are compile-time constants
(one-hot vectors) and op nodes at pass 0 are zero, so the per-row
worklists are tiny (mean ~5 updates/row for the benchmark distribution).
The host precomputes integer worklists / gather indices; the device does
all floating-point math.

Device pipeline per pass: ap_gather pulls child value vectors out of
per-row-group SBUF value buffers (rows are binned into 8 groups so all 8
GPSIMD Q7 cores gather concurrently); small DMAs concatenate the
group-blocked gather output into contraction layout; TensorE builds the
replicated operands and the bilinear contraction; softmax runs as
exp (ScalarE) + ones-matmul reduction (TensorE) + reciprocal (VectorE);
all three per-op softmax results are stored so op selection folds into
the next pass's gather indices.  Pass-1 inputs are constants and are
shipped from the host directly, skipping one gather.

Sharding: pure data parallel over the batch dim (B=2048 -> 256 rows on
each of the 8 NeuronCores), per the sharding hint.  No collectives are
needed for the forward pass.
"""

import sys
from contextlib import ExitStack

import numpy as np

sys.path.insert(0, "/opt/trn_rl_repo")

import concourse.bass as bass
import concourse.tile as tile
from concourse import bacc, mybir
from concourse.bass_utils import run_bass_kernel_spmd

B, N = 2048, 1023
NI, NO, NP = 10, 3, 10  # n_ints, n_ops, n_passes
NCORES = 8
RPC = B // NCORES  # rows per core
NG = 8  # row groups per core (one per Q7 core / 16-partition block)
ZSLOT = NI  # value-buffer slot holding the zero vector
NCONST = NI + 1  # slots 0..9 = one-hot e_k, slot 10 = zeros
CHUNK = 448  # free-dim chunk for the compute pipeline (PSUM/matmul limits)

TRACE = False  # set True (e.g. from test.py) to profile the HW run
LAST_RESULTS = None  # BassKernelResults of the last run (exec_time_ns etc.)


def _plan(cats, ops, lits, left, right, mask):
    """Integer-only preprocessing: worklists, group binning, gather indices."""
    left = np.clip(left.astype(np.int64), 0, N - 1)
    right = np.clip(right.astype(np.int64), 0, N - 1)
    opsc = np.clip(ops.astype(np.int64), 0, NO - 1)
    litsc = np.clip(lits.astype(np.int64), 0, NI - 1)
    m = mask.astype(bool)
    is_lit = (cats == 0) & m
    is_opa = (cats == 1) & m
    const_slot = np.where(is_lit, litsc, ZSLOT)

    # Worklists W[p]: (row, node) updates needed at pass p.
    Wr = [None] * (NP + 1)
    Wn = [None] * (NP + 1)
    r10 = np.nonzero(cats[:, 0] == 1)[0].astype(np.int64)
    Wr[NP], Wn[NP] = r10, np.zeros(len(r10), np.int64)
    need = np.zeros((B, N), bool)
    for p in range(NP, 1, -1):
        r, n = Wr[p], Wn[p]
        cr = np.concatenate([r, r])
        cn = np.concatenate([left[r, n], right[r, n]])
        keep = is_opa[cr, cn]
        need[:] = False
        need[cr[keep], cn[keep]] = True
        rr, nn = np.nonzero(need)
        Wr[p - 1], Wn[p - 1] = rr.astype(np.int64), nn.astype(np.int64)

    # Bin rows into NG groups per core, balancing total updates per group.
    weight = np.zeros(B, np.int64)
    for p in range(1, NP + 1):
        np.add.at(weight, Wr[p], 1)
    G = np.zeros(B, np.int64)
    for c in range(NCORES):
        rows = np.arange(c * RPC, (c + 1) * RPC)
        order = rows[np.argsort(-weight[rows], kind="stable")]
        load = np.zeros(NG, np.int64)
        for rr_ in order:
            g = int(load.argmin())
            G[rr_] = g
            load[g] += weight[rr_]

    # Per-pass group-local ids and padded per-group size Q_p.
    Qp = np.zeros(NP + 1, np.int64)
    gid = [None] * (NP + 1)
    for p in range(1, NP + 1):
        r = Wr[p]
        core = r // RPC
        grp = G[r]
        key = core * NG + grp
        order = np.argsort(key, kind="stable")
        ks = key[order]
        u = np.arange(len(ks), dtype=np.int64)
        if len(ks):
            first = np.r_[True, ks[1:] != ks[:-1]]
            seg_idx = np.nonzero(first)[0]
            u = u - seg_idx[np.cumsum(first) - 1]
        ul = np.empty(len(ks), np.int64)
        ul[order] = u
        cnt = np.bincount(key, minlength=NCORES * NG) if len(r) else np.zeros(NCORES * NG, np.int64)
        mx = int(cnt.max()) if len(r) else 0
        Qp[p] = max(8, -(-mx // 8) * 8)  # multiple of 8 -> num_idxs % 16 == 0
        gid[p] = (core, grp, ul)

    # Buffer slot bases (group-local numbering); passes 1..NP-1 store 3 slots/update.
    base = np.zeros(NP + 1, np.int64)
    base[1] = NCONST
    for p in range(2, NP + 1):
        base[p] = base[p - 1] + 3 * Qp[p - 1]
    S = int(base[NP - 1] + 3 * Qp[NP - 1])
    assert S <= 32000, f"value buffer too large for int16 gather indices: {S}"

    idx_wrapped = []
    Ftot = 0
    slot_prev = np.full((B, N), -1, np.int64)
    lr1 = None
    for p in range(1, NP + 1):
        r, n = Wr[p], Wn[p]
        core, grp, ul = gid[p]
        lch, rch = left[r, n], right[r, n]
        if p == 1:
            lidx = const_slot[r, lch]
            ridx = const_slot[r, rch]
        else:
            lidx = np.where(is_opa[r, lch],
                            base[p - 1] + 3 * slot_prev[r, lch] + opsc[r, lch],
                            const_slot[r, lch])
            ridx = np.where(is_opa[r, rch],
                            base[p - 1] + 3 * slot_prev[r, rch] + opsc[r, rch],
                            const_slot[r, rch])
        Q = int(Qp[p])
        arr = np.full((NCORES, NG, 2 * Q), ZSLOT, np.int64)
        arr[core, grp, ul] = lidx
        arr[core, grp, Q + ul] = ridx
        if p == 1:
            # pass-1 inputs are constants; ship lr1 from host (skip the gather).
            # lr10 layout: (10, 2*NG*Q): l half col g*Q+u ; r half col NG*Q+g*Q+u
            eyeext = np.concatenate([np.eye(NI, dtype=np.float32),
                                     np.zeros((NI, 1), np.float32)], axis=1)
            cols = arr.reshape(NCORES, NG, 2, Q).transpose(0, 2, 1, 3).reshape(NCORES, 2 * NG * Q)
            lr1 = np.ascontiguousarray(eyeext[:, cols].transpose(1, 0, 2))  # (NCORES, 10, 2*NG*Q)
        else:
            F = -(-2 * Q // 16)
            F += F & 1  # 4-byte-aligned idx slices (ucode reads dwords)
            tmp = np.full((NCORES, NG, F * 16), ZSLOT, np.int64)
            tmp[:, :, : 2 * Q] = arr
            w = tmp.reshape(NCORES, NG, F, 16).transpose(0, 1, 3, 2).reshape(NCORES, NG * 16, F)
            idx_wrapped.append(w.astype(np.int16))
            Ftot += F
        if p < NP:
            slot_prev = np.full((B, N), -1, np.int64)
            slot_prev[r, n] = ul

    idx_full = np.concatenate(idx_wrapped, axis=2)  # (NCORES, 128, Ftot)

    return dict(
        Qp=Qp, base=base, S=S, idx=idx_full, Ftot=Ftot, lr1=lr1,
        r10=r10, gid10=gid[NP],
        opsc=opsc, litsc=litsc, is_lit=is_lit, m=m, G=G,
    )


_CUR_BASE = None


def _build_nc(S, Qp, Ftot):
    f32 = mybir.dt.float32
    Q10 = int(Qp[NP])
    PT10 = NG * Q10
    nc = bacc.Bacc(None)
    consts = nc.dram_tensor("consts", [NI, NCONST], f32, kind="ExternalInput")
    wmat = nc.dram_tensor("wmat", [100, 74], f32, kind="ExternalInput")
    repl = nc.dram_tensor("repl", [NI, 100], f32, kind="ExternalInput")
    reprm = nc.dram_tensor("reprm", [NI, 100], f32, kind="ExternalInput")
    oblk = nc.dram_tensor("oblk", [74, NO], f32, kind="ExternalInput")
    oblk2 = nc.dram_tensor("oblk2", [NO, 74], f32, kind="ExternalInput")
    idx_in = nc.dram_tensor("idx", [128, Ftot], mybir.dt.int16, kind="ExternalInput")
    PT1 = NG * int(Qp[1])
    lr1_in = nc.dram_tensor("lr1", [NI, 2 * PT1], f32, kind="ExternalInput")
    outz = nc.dram_tensor("outz", [74, PT10], f32, kind="ExternalOutput")

    with ExitStack() as ctx:
        tc = ctx.enter_context(tile.TileContext(nc))
        singles = ctx.enter_context(tc.tile_pool(name="singles", bufs=1))
        work = ctx.enter_context(tc.tile_pool(name="work", bufs=2))
        psum = ctx.enter_context(tc.tile_pool(name="psum", bufs=1, space="PSUM"))
        lrpool = ctx.enter_context(tc.tile_pool(name="lrpool", bufs=1))

        buf = singles.tile([128, S], f32)
        nc.vector.memset(buf[:, :], 0.0)
        for g in range(NG):
            nc.sync.dma_start(out=buf[16 * g : 16 * g + NI, 0:NCONST], in_=consts[:, :])
        w_sb = singles.tile([100, 74], f32)
        nc.sync.dma_start(out=w_sb[:, :], in_=wmat[:, :])
        repl_sb = singles.tile([NI, 100], f32)
        nc.sync.dma_start(out=repl_sb[:, :], in_=repl[:, :])
        reprm_sb = singles.tile([NI, 100], f32)
        nc.sync.dma_start(out=reprm_sb[:, :], in_=reprm[:, :])
        oblk_sb = singles.tile([74, NO], f32)
        nc.sync.dma_start(out=oblk_sb[:, :], in_=oblk[:, :])
        oblk2_sb = singles.tile([NO, 74], f32)
        nc.sync.dma_start(out=oblk2_sb[:, :], in_=oblk2[:, :])
        idx_sb = singles.tile([128, Ftot], mybir.dt.int16)
        nc.sync.dma_start(out=idx_sb[:, :], in_=idx_in[:, :])

        foff = 0
        for p in range(1, NP + 1):
            Q = int(Qp[p])
            PT = NG * Q
            lr10 = lrpool.tile([NI, 2 * PT], f32, tag=f"lr10_{p}")
            if p == 1:
                nc.sync.dma_start(out=lr10[:, :], in_=lr1_in[:, :])
            else:
                F = -(-2 * Q // 16)
                F += F & 1
                lrg = lrpool.tile([128, 2 * Q], f32, tag=f"lrg{p}")
                nc.gpsimd.ap_gather(
                    out_ap=lrg[:, :],
                    in_ap=buf[:, :],
                    idxs_ap=idx_sb[:, foff : foff + F],
                    channels=128,
                    num_elems=S,
                    d=1,
                    num_idxs=2 * Q,
                )
                foff += F
                # concat groups: lr10[i, h*PT + g*Q + u] = lrg[16g+i, h*Q + u]
                for g in range(NG):
                    src = lrg[16 * g : 16 * g + NI, :].rearrange("i (h u) -> i h u", h=2)
                    dst = lr10[:, :].rearrange("i (h gg u) -> i h gg u", h=2, gg=NG)[:, :, g, :]
                    nc.sync.dma_start(out=dst, in_=src)
            for c0 in range(0, PT, CHUNK):
                cw = min(CHUNK, PT - c0)
                ps_l = psum.tile([100, cw], f32, tag="ps_l")
                nc.tensor.matmul(ps_l[:, :], repl_sb[:, :], lr10[:, c0 : c0 + cw],
                                 start=True, stop=True)
                ps_r = psum.tile([100, cw], f32, tag="ps_r")
                nc.tensor.matmul(ps_r[:, :], reprm_sb[:, :], lr10[:, PT + c0 : PT + c0 + cw],
                                 start=True, stop=True)
                lrep_sb = work.tile([100, cw], f32, tag="lrep_sb")
                nc.vector.tensor_copy(lrep_sb[:, :], ps_l[:, :])
                outer = work.tile([100, cw], f32, tag="outer")
                nc.vector.tensor_mul(outer[:, :], lrep_sb[:, :], ps_r[:, :])
                ps_z = psum.tile([74, cw], f32, tag="ps_z")
                nc.tensor.matmul(ps_z[:, :], w_sb[:, :], outer[:, :], start=True, stop=True)
                if p == NP:
                    zsb = work.tile([74, cw], f32, tag="zsb")
                    nc.scalar.copy(zsb[:, :], ps_z[:, :])
                    nc.sync.dma_start(out=outz[:, c0 : c0 + cw], in_=zsb[:, :])
                    continue
                e = work.tile([74, cw], f32, tag="e")
                nc.scalar.activation(e[:, :], ps_z[:, :], mybir.ActivationFunctionType.Exp)
                ps_z3 = psum.tile([NO, cw], f32, tag="ps_z3")
                nc.tensor.matmul(ps_z3[:, :], oblk_sb[:, :], e[:, :], start=True, stop=True)
                rz = work.tile([NO, cw], f32, tag="rz")
                nc.vector.reciprocal(rz[:, :], ps_z3[:, :])
                ps_rz = psum.tile([74, cw], f32, tag="ps_rz")
                nc.tensor.matmul(ps_rz[:, :], oblk2_sb[:, :], rz[:, :], start=True, stop=True)
                st = work.tile([NI, 3 * cw], f32, tag="st")
                for o in range(NO):
                    nc.vector.tensor_mul(
                        st[:, o : 3 * cw : 3],
                        e[o * 32 : o * 32 + NI, :],
                        ps_rz[o * 32 : o * 32 + NI, :],
                    )
                # scatter back: buf[16g+k, b0+3u+o] = st[k, 3*(g*Q+u)+o]
                b0 = int(_CUR_BASE[p])
                for g in range(NG):
                    nc.sync.dma_start(
                        out=buf[16 * g : 16 * g + NI, b0 : b0 + 3 * Q],
                        in_=st[:, 3 * g * Q : 3 * (g + 1) * Q],
                    )
    nc.finalize()
    return nc


def kernel(op_table, cats, ops, lits, left, right, mask):
    global _CUR_BASE, LAST_RESULTS
    op_table = np.asarray(op_table, np.float32)
    plan = _plan(np.asarray(cats), np.asarray(ops), np.asarray(lits),
                 np.asarray(left), np.asarray(right), np.asarray(mask))
    Qp, base, S, Ftot = plan["Qp"], plan["base"], plan["S"], plan["Ftot"]
    _CUR_BASE = base
    assert NG * int(max(Qp[1:])) <= CHUNK, f"chunking not supported: {Qp}"

    nc = _build_nc(S, Qp, Ftot)

    consts = np.concatenate([np.eye(NI, dtype=np.float32),
                             np.zeros((NI, 1), np.float32)], axis=1)
    wmat = np.zeros((100, 74), np.float32)
    w30 = op_table.transpose(1, 2, 0, 3).reshape(100, 30)
    oblk = np.zeros((74, NO), np.float32)
    oblk2 = np.zeros((NO, 74), np.float32)
    for o in range(NO):
        wmat[:, o * 32 : o * 32 + NI] = w30[:, o * NI : (o + 1) * NI]
        oblk[o * 32 : o * 32 + NI, o] = 1.0
        oblk2[o, o * 32 : o * 32 + NI] = 1.0
    repl = np.kron(np.eye(NI), np.ones((1, NI))).astype(np.float32)
    reprm = np.tile(np.eye(NI), (1, NI)).astype(np.float32)

    in_maps = []
    for c in range(NCORES):
        in_maps.append({
            "consts": consts, "wmat": wmat, "repl": repl, "reprm": reprm,
            "oblk": oblk, "oblk2": oblk2,
            "idx": np.ascontiguousarray(plan["idx"][c]),
            "lr1": np.ascontiguousarray(plan["lr1"][c]),
        })

    res = run_bass_kernel_spmd(nc, in_maps, list(range(NCORES)), trace=TRACE)
    LAST_RESULTS = res

    # Assemble the full (B, NI) output on the host (index selection only).
    out = np.zeros((B, NI), np.float32)
    litsc, is_lit = plan["litsc"], plan["is_lit"]
    lit_rows = np.nonzero(cats[:, 0] == 0)[0]
    lr_active = is_lit[lit_rows, 0]
    oh = 10.0 * np.eye(NI, dtype=np.float32)[litsc[lit_rows, 0]]
    out[lit_rows] = np.where(lr_active[:, None], oh, 0.0)

    r10, opsc = plan["r10"], plan["opsc"]
    core10, grp10, ul10 = plan["gid10"]
    Q10 = int(Qp[NP])
    cols = grp10 * Q10 + ul10
    for c in range(NCORES):
        z = np.asarray(res.results[c]["outz"])  # (74, PT10)
        selmask = core10 == c
        rows = r10[selmask]
        cc = cols[selmask]
        o = opsc[rows, 0]
        zc = z[:, cc]
        sel = np.stack([zc[i * 32 : i * 32 + NI, :] for i in range(NO)])
        out[rows] = sel[o, :, np.arange(len(rows))]
    return out



# revision 2
# speedup vs baseline: 2.6970x; 2.6970x over previous
"""Trainium2 Bass kernel for nn_CruxMiniCircuit (gnn_message_passing).

Reference semantics: B independent rows; each row is a circuit of N nodes
(literal nodes hold a fixed one-hot distribution over 10 ints, op nodes
combine left/right child distributions through a per-op bilinear table
followed by softmax).  The reference runs 10 synchronous passes over all
nodes and returns only the root (node 0) logits per row.

Key observation: the output depends only on node 0's dependency cone
unrolled 10 passes deep.  Literal children are compile-time constants
(one-hot vectors) and op nodes at pass 0 are zero, so the per-row
worklists are tiny (mean ~5 updates/row for the benchmark distribution).
The host precomputes integer worklists / gather indices; the device does
all floating-point math.

Device pipeline per pass (v2): rows are binned into NG=4 groups per core
placed at quadrant-aligned partitions (32g..32g+9) so that the gather
output can be rearranged into contraction layout with compute-engine
copies (DVE/ActE support cross-quadrant partition moves) instead of the
serialized ~750ns HWDGE DMAs the v1 kernel used — those DMAs were ~47%
of each pass.  The bilinear contraction runs in bf16 (single-pump PE
matmuls), softmax denominators use one fused block-sum-broadcast matmul
(pmat) plus a fast-approx reciprocal, and op selection folds into the
next pass's gather indices.  A dummy ap_gather at t=0 preloads the
GPSIMD IRAM so the ~5us ucode load overlaps the constant DMAs.

Sharding: pure data parallel over the batch dim (B=2048 -> 256 rows on
each of the 8 NeuronCores), per the sharding hint.  No collectives are
needed for the forward pass.
"""

import sys
from contextlib import ExitStack

import numpy as np

sys.path.insert(0, "/opt/trn_rl_repo")

import ml_dtypes

import concourse.bass as bass
import concourse.tile as tile
from concourse import bacc, mybir
from concourse.bass_utils import run_bass_kernel_spmd

B, N = 2048, 1023
NI, NO, NP = 10, 3, 10  # n_ints, n_ops, n_passes
NCORES = 8
RPC = B // NCORES  # rows per core
NG = 4  # row groups per core, one per quadrant (partitions 32g..32g+9)
ZSLOT = NI  # value-buffer slot holding the zero vector
NCONST = NI + 1  # slots 0..9 = one-hot e_k, slot 10 = zeros
CHUNK = 448  # free-dim chunk for the compute pipeline (PSUM/matmul limits)

TRACE = False  # set True (e.g. from test.py) to profile the HW run
LAST_RESULTS = None  # BassKernelResults of the last run (exec_time_ns etc.)

BF = mybir.dt.bfloat16
F32 = mybir.dt.float32


def _plan(cats, ops, lits, left, right, mask):
    """Integer-only preprocessing: worklists, group binning, gather indices."""
    left = np.clip(left.astype(np.int64), 0, N - 1)
    right = np.clip(right.astype(np.int64), 0, N - 1)
    opsc = np.clip(ops.astype(np.int64), 0, NO - 1)
    litsc = np.clip(lits.astype(np.int64), 0, NI - 1)
    m = mask.astype(bool)
    is_lit = (cats == 0) & m
    is_opa = (cats == 1) & m
    const_slot = np.where(is_lit, litsc, ZSLOT)

    # Worklists W[p]: (row, node) updates needed at pass p.
    Wr = [None] * (NP + 1)
    Wn = [None] * (NP + 1)
    r10 = np.nonzero(cats[:, 0] == 1)[0].astype(np.int64)
    Wr[NP], Wn[NP] = r10, np.zeros(len(r10), np.int64)
    need = np.zeros((B, N), bool)
    for p in range(NP, 1, -1):
        r, n = Wr[p], Wn[p]
        cr = np.concatenate([r, r])
        cn = np.concatenate([left[r, n], right[r, n]])
        keep = is_opa[cr, cn]
        need[:] = False
        need[cr[keep], cn[keep]] = True
        rr, nn = np.nonzero(need)
        Wr[p - 1], Wn[p - 1] = rr.astype(np.int64), nn.astype(np.int64)

    # Bin rows into NG groups per core, balancing total updates per group.
    weight = np.zeros(B, np.int64)
    for p in range(1, NP + 1):
        np.add.at(weight, Wr[p], 1)
    G = np.zeros(B, np.int64)
    for c in range(NCORES):
        rows = np.arange(c * RPC, (c + 1) * RPC)
        order = rows[np.argsort(-weight[rows], kind="stable")]
        load = np.zeros(NG, np.int64)
        for rr_ in order:
            g = int(load.argmin())
            G[rr_] = g
            load[g] += weight[rr_]

    # Per-pass group-local ids and padded per-group size Q_p.
    Qp = np.zeros(NP + 1, np.int64)
    gid = [None] * (NP + 1)
    for p in range(1, NP + 1):
        r = Wr[p]
        core = r // RPC
        grp = G[r]
        key = core * NG + grp
        order = np.argsort(key, kind="stable")
        ks = key[order]
        u = np.arange(len(ks), dtype=np.int64)
        if len(ks):
            first = np.r_[True, ks[1:] != ks[:-1]]
            seg_idx = np.nonzero(first)[0]
            u = u - seg_idx[np.cumsum(first) - 1]
        ul = np.empty(len(ks), np.int64)
        ul[order] = u
        cnt = np.bincount(key, minlength=NCORES * NG) if len(r) else np.zeros(NCORES * NG, np.int64)
        mx = int(cnt.max()) if len(r) else 0
        Qp[p] = max(8, -(-mx // 8) * 8)  # multiple of 8 -> num_idxs % 16 == 0
        gid[p] = (core, grp, ul)

    # Buffer slot bases; pass-p results stored o-major: base[p] + o*Qp[p] + u.
    base = np.zeros(NP + 1, np.int64)
    base[1] = NCONST
    for p in range(2, NP + 1):
        base[p] = base[p - 1] + 3 * Qp[p - 1]
    S = int(base[NP - 1] + 3 * Qp[NP - 1])
    assert S <= 32000, f"value buffer too large for int16 gather indices: {S}"

    idx_wrapped = []
    Ftot = 0
    slot_prev = np.full((B, N), -1, np.int64)
    lr1 = None
    for p in range(1, NP + 1):
        r, n = Wr[p], Wn[p]
        core, grp, ul = gid[p]
        lch, rch = left[r, n], right[r, n]
        if p == 1:
            lidx = const_slot[r, lch]
            ridx = const_slot[r, rch]
        else:
            lidx = np.where(is_opa[r, lch],
                            base[p - 1] + opsc[r, lch] * Qp[p - 1] + slot_prev[r, lch],
                            const_slot[r, lch])
            ridx = np.where(is_opa[r, rch],
                            base[p - 1] + opsc[r, rch] * Qp[p - 1] + slot_prev[r, rch],
                            const_slot[r, rch])
        Q = int(Qp[p])
        arr = np.full((NCORES, NG, 2 * Q), ZSLOT, np.int64)
        arr[core, grp, ul] = lidx
        arr[core, grp, Q + ul] = ridx
        if p == 1:
            # pass-1 inputs are constants; ship lr1 from host (skip the gather).
            # lr1 layout: (10, 2*NG*Q): l half col g*Q+u ; r half col NG*Q+g*Q+u
            eyeext = np.concatenate([np.eye(NI, dtype=np.float32),
                                     np.zeros((NI, 1), np.float32)], axis=1)
            cols = arr.reshape(NCORES, NG, 2, Q).transpose(0, 2, 1, 3).reshape(NCORES, 2 * NG * Q)
            lr1 = np.ascontiguousarray(eyeext[:, cols].transpose(1, 0, 2))  # (NCORES, 10, 2*NG*Q)
        else:
            F = -(-2 * Q // 16)
            F += F & 1  # 4-byte-aligned idx slices (ucode reads dwords)
            # 8 16-partition blocks per core; group g's rows live in block 2g
            # (partitions 32g..32g+15); odd blocks gather ZSLOT junk.
            tmp = np.full((NCORES, 2 * NG, F * 16), ZSLOT, np.int64)
            tmp[:, 0::2, : 2 * Q] = arr
            w = tmp.reshape(NCORES, 2 * NG, F, 16).transpose(0, 1, 3, 2).reshape(NCORES, 128, F)
            idx_wrapped.append(w.astype(np.int16))
            Ftot += F
        if p < NP:
            slot_prev = np.full((B, N), -1, np.int64)
            slot_prev[r, n] = ul

    idx_full = np.concatenate(idx_wrapped, axis=2)  # (NCORES, 128, Ftot)

    return dict(
        Qp=Qp, base=base, S=S, idx=idx_full, Ftot=Ftot, lr1=lr1,
        r10=r10, gid10=gid[NP],
        opsc=opsc, litsc=litsc, is_lit=is_lit, m=m, G=G,
    )


_CUR_BASE = None


def _build_nc(S, Qp, Ftot):
    Q10 = int(Qp[NP])
    PT10 = NG * Q10
    Qmax = int(max(Qp[1:]))
    PTmax = NG * Qmax
    nc = bacc.Bacc(None)
    consts = nc.dram_tensor("consts", [128, NCONST], F32, kind="ExternalInput")
    wmat = nc.dram_tensor("wmat", [100, 74], BF, kind="ExternalInput")
    repl = nc.dram_tensor("repl", [NI, 100], BF, kind="ExternalInput")
    reprm = nc.dram_tensor("reprm", [NI, 100], BF, kind="ExternalInput")
    pmat = nc.dram_tensor("pmat", [74, 74], BF, kind="ExternalInput")
    idx_in = nc.dram_tensor("idx", [128, Ftot], mybir.dt.int16, kind="ExternalInput")
    PT1 = NG * int(Qp[1])
    lr1_in = nc.dram_tensor("lr1", [NI, 2 * PT1], BF, kind="ExternalInput")
    outz = nc.dram_tensor("outz", [74, PT10], F32, kind="ExternalOutput")

    with ExitStack() as ctx:
        tc = ctx.enter_context(tile.TileContext(nc))
        singles = ctx.enter_context(tc.tile_pool(name="singles", bufs=1))
        psum = ctx.enter_context(tc.tile_pool(name="psum", bufs=1, space="PSUM"))

        # --- GPSIMD IRAM preload: a dummy gather, first instr on the queue.
        dummy_src = singles.tile([128, 16], F32)
        dummy_idx = singles.tile([128, 2], mybir.dt.int16)
        dummy_out = singles.tile([128, 16], F32)
        nc.vector.memset(dummy_src[:, :], 0.0)
        nc.vector.memset(dummy_idx[:, :], 0)
        nc.gpsimd.ap_gather(
            out_ap=dummy_out[:, :], in_ap=dummy_src[:, :], idxs_ap=dummy_idx[:, :],
            channels=128, num_elems=16, d=1, num_idxs=16,
        )

        # --- constants
        buf = singles.tile([128, S], F32)
        nc.vector.memset(buf[:, :], 0.0)
        nc.sync.dma_start(out=buf[:, 0:NCONST], in_=consts[:, :])
        w_sb = singles.tile([100, 74], BF)
        nc.sync.dma_start(out=w_sb[:, :], in_=wmat[:, :])
        repl_sb = singles.tile([NI, 100], BF)
        nc.scalar.dma_start(out=repl_sb[:, :], in_=repl[:, :])
        reprm_sb = singles.tile([NI, 100], BF)
        nc.scalar.dma_start(out=reprm_sb[:, :], in_=reprm[:, :])
        pmat_sb = singles.tile([74, 74], BF)
        nc.sync.dma_start(out=pmat_sb[:, :], in_=pmat[:, :])
        idx_sb = singles.tile([128, Ftot], mybir.dt.int16)
        nc.sync.dma_start(out=idx_sb[:, :], in_=idx_in[:, :])
        lr1_sb = singles.tile([NI, 2 * PT1], BF)
        nc.scalar.dma_start(out=lr1_sb[:, :], in_=lr1_in[:, :])

        # --- work tiles (max size, sliced per pass)
        lrg = singles.tile([128, 2 * Qmax], F32)
        lr10 = singles.tile([NI, 2 * PTmax], BF)
        lsb = singles.tile([100, PTmax], F32)
        outer = singles.tile([100, PTmax], BF)
        e = singles.tile([74, PTmax], BF)
        rz = singles.tile([74, PTmax], F32)
        st = singles.tile([NI, 3 * PTmax], F32)
        zsb = singles.tile([74, PT10], F32)
        ps_l = psum.tile([100, PTmax], F32, tag="ps_l")
        ps_r = psum.tile([100, PTmax], F32, tag="ps_r")
        ps_z = psum.tile([74, PTmax], F32, tag="ps_z")
        ps_bc = psum.tile([74, PTmax], F32, tag="ps_bc")

        foff = 0
        for p in range(1, NP + 1):
            Q = int(Qp[p])
            PT = NG * Q
            if p == 1:
                lr = lr1_sb
            else:
                F = -(-2 * Q // 16)
                F += F & 1
                nc.gpsimd.ap_gather(
                    out_ap=lrg[:, 0 : 2 * Q],
                    in_ap=buf[:, :],
                    idxs_ap=idx_sb[:, foff : foff + F],
                    channels=128,
                    num_elems=S,
                    d=1,
                    num_idxs=2 * Q,
                )
                foff += F
                # concat groups: lr10[i, h*PT + g*Q + u] = lrg[32g+i, h*Q + u]
                lr = lr10[:, 0 : 2 * PT]
                dst = lr.rearrange("i (h gg u) -> i h gg u", h=2, gg=NG)
                for g in range(NG):
                    src = lrg[32 * g : 32 * g + NI, 0 : 2 * Q].rearrange(
                        "i (h u) -> i h u", h=2)
                    eng = nc.vector if g % 2 == 0 else nc.scalar
                    if g % 2 == 0:
                        nc.vector.tensor_copy(dst[:, :, g, :], src)
                    else:
                        nc.scalar.copy(dst[:, :, g, :], src)
            nc.tensor.matmul(ps_l[:, 0:PT], repl_sb[:, :], lr[:, 0:PT],
                             start=True, stop=True)
            nc.scalar.copy(lsb[:, 0:PT], ps_l[:, 0:PT])
            nc.tensor.matmul(ps_r[:, 0:PT], reprm_sb[:, :], lr[:, PT : 2 * PT],
                             start=True, stop=True)
            nc.vector.tensor_mul(outer[:, 0:PT], lsb[:, 0:PT], ps_r[:, 0:PT])
            nc.tensor.matmul(ps_z[:, 0:PT], w_sb[:, :], outer[:, 0:PT],
                             start=True, stop=True)
            if p == NP:
                nc.scalar.copy(zsb[:, :], ps_z[:, 0:PT])
                nc.sync.dma_start(out=outz[:, :], in_=zsb[:, :])
                continue
            nc.scalar.activation(e[:, 0:PT], ps_z[:, 0:PT],
                                 mybir.ActivationFunctionType.Exp)
            nc.tensor.matmul(ps_bc[:, 0:PT], pmat_sb[:, :], e[:, 0:PT],
                             start=True, stop=True)
            nc.vector.reciprocal_approx_fast(out=rz[:, 0:PT], in_=ps_bc[:, 0:PT])
            for o in range(NO):
                nc.vector.tensor_mul(
                    st[:, o * PT : (o + 1) * PT],
                    e[32 * o : 32 * o + NI, 0:PT],
                    rz[32 * o : 32 * o + NI, 0:PT],
                )
            # scatter back: buf[32g+i, b0 + o*Q + u] = st[i, o*PT + g*Q + u]
            b0 = int(_CUR_BASE[p])
            stv = st[:, 0 : 3 * PT].rearrange("i (o c) -> i o c", o=3)
            for g in range(NG):
                dst = buf[32 * g : 32 * g + NI, b0 : b0 + 3 * Q].rearrange(
                    "i (o u) -> i o u", o=3)
                src = stv[:, :, g * Q : (g + 1) * Q]
                if g % 2 == 0:
                    nc.vector.tensor_copy(dst, src)
                else:
                    nc.scalar.copy(dst, src)
    nc.finalize()
    return nc


def kernel(op_table, cats, ops, lits, left, right, mask):
    global _CUR_BASE, LAST_RESULTS
    op_table = np.asarray(op_table, np.float32)
    plan = _plan(np.asarray(cats), np.asarray(ops), np.asarray(lits),
                 np.asarray(left), np.asarray(right), np.asarray(mask))
    Qp, base, S, Ftot = plan["Qp"], plan["base"], plan["S"], plan["Ftot"]
    _CUR_BASE = base
    assert NG * int(max(Qp[1:])) <= CHUNK, f"chunking not supported: {Qp}"

    nc = _build_nc(S, Qp, Ftot)

    eyeext = np.concatenate([np.eye(NI, dtype=np.float32),
                             np.zeros((NI, 1), np.float32)], axis=1)
    consts = np.zeros((128, NCONST), np.float32)
    for g in range(NG):
        consts[32 * g : 32 * g + NI, :] = eyeext
    wmat = np.zeros((100, 74), np.float32)
    w30 = op_table.transpose(1, 2, 0, 3).reshape(100, 30)
    pmat = np.zeros((74, 74), np.float32)
    for o in range(NO):
        wmat[:, o * 32 : o * 32 + NI] = w30[:, o * NI : (o + 1) * NI]
        pmat[o * 32 : o * 32 + NI, o * 32 : o * 32 + NI] = 1.0
    repl = np.kron(np.eye(NI), np.ones((1, NI))).astype(np.float32)
    reprm = np.tile(np.eye(NI), (1, NI)).astype(np.float32)

    bf = ml_dtypes.bfloat16
    in_maps = []
    for c in range(NCORES):
        in_maps.append({
            "consts": consts,
            "wmat": wmat.astype(bf), "repl": repl.astype(bf),
            "reprm": reprm.astype(bf), "pmat": pmat.astype(bf),
            "idx": np.ascontiguousarray(plan["idx"][c]),
            "lr1": np.ascontiguousarray(plan["lr1"][c].astype(bf)),
        })

    res = run_bass_kernel_spmd(nc, in_maps, list(range(NCORES)), trace=TRACE)
    LAST_RESULTS = res

    # Assemble the full (B, NI) output on the host (index selection only).
    out = np.zeros((B, NI), np.float32)
    litsc, is_lit = plan["litsc"], plan["is_lit"]
    lit_rows = np.nonzero(cats[:, 0] == 0)[0]
    lr_active = is_lit[lit_rows, 0]
    oh = 10.0 * np.eye(NI, dtype=np.float32)[litsc[lit_rows, 0]]
    out[lit_rows] = np.where(lr_active[:, None], oh, 0.0)

    r10, opsc = plan["r10"], plan["opsc"]
    core10, grp10, ul10 = plan["gid10"]
    Q10 = int(Qp[NP])
    cols = grp10 * Q10 + ul10
    for c in range(NCORES):
        z = np.asarray(res.results[c]["outz"])  # (74, PT10)
        selmask = core10 == c
        rows = r10[selmask]
        cc = cols[selmask]
        o = opsc[rows, 0]
        zc = z[:, cc]
        sel = np.stack([zc[i * 32 : i * 32 + NI, :] for i in range(NO)])
        out[rows] = sel[o, :, np.arange(len(rows))]
    return out


# revision 7
# speedup vs baseline: 4.7292x; 1.7535x over previous
"""Trainium2 Bass kernel for nn_CruxMiniCircuit (gnn_message_passing).

Reference semantics: B independent rows; each row is a circuit of N nodes
(literal nodes hold a fixed one-hot distribution over 10 ints, op nodes
combine left/right child distributions through a per-op bilinear table
followed by softmax).  The reference runs 10 synchronous passes over all
nodes and returns only the root (node 0) logits per row.

Key observation: the output depends only on node 0's dependency cone
unrolled 10 passes deep.  Literal children are compile-time constants
(one-hot vectors) and op nodes at pass 0 are zero, so the per-row
worklists are tiny (mean ~5 updates/row for the benchmark distribution).
The host precomputes integer worklists / gather indices; the device does
all floating-point math.

v3 device pipeline ("all-matmul", zero GPSIMD): the per-pass state is
kept TRANSPOSED in SBUF as stT3[slot, 30*NG] (slots on partitions, all
three op-variant softmaxes on the free dim), so the child-value gather
becomes 4 small selection matmuls against constant one-hot matrices
(host-built, DMA'd at startup) and the op-variant choice becomes one
elementwise multiply with a host-built 0/1 mask.  The bilinear logits
are produced TRANSPOSED (zT[item, 30]) by using the outer-product tile
as the matmul stationary operand, which lets softmax run along the free
dim: exp -> grouped free-dim reduce -> fast-approx reciprocal ->
stride-0 broadcast multiply.  No ap_gather (whose ~2.7us/invocation
hidden dispatch stall dominated v2), no scatter, no value buffer.
Everything is bf16 matmuls (single-pump PE) + f32 softmax.

Sharding: pure data parallel over the batch dim (B=2048 -> 256 rows on
each of the 8 NeuronCores), per the sharding hint.  No collectives are
needed for the forward pass.
"""

import sys
from contextlib import ExitStack

import numpy as np

sys.path.insert(0, "/opt/trn_rl_repo")

import ml_dtypes

import concourse.bass as bass
import concourse.tile as tile
from concourse import bacc, mybir
from concourse.bass_utils import run_bass_kernel_spmd

B, N = 2048, 1023
NI, NO, NP = 10, 3, 10  # n_ints, n_ops, n_passes
NCORES = 8
RPC = B // NCORES  # rows per core
NG = 4  # groups per core: column blocks of stT3 / outer / zT
CROW = 117  # stT3 row where the 11 constant one-hot slots start
QLIM = CROW  # value slots per group must fit below the const rows

TRACE = False
LAST_RESULTS = None

BF = mybir.dt.bfloat16
F32 = mybir.dt.float32


def _plan(cats, ops, lits, left, right, mask):
    """Integer-only preprocessing: worklists, binning, Sel/M30 matrices."""
    left = np.clip(left.astype(np.int64), 0, N - 1)
    right = np.clip(right.astype(np.int64), 0, N - 1)
    opsc = np.clip(ops.astype(np.int64), 0, NO - 1)
    litsc = np.clip(lits.astype(np.int64), 0, NI - 1)
    m = mask.astype(bool)
    is_lit = (cats == 0) & m
    is_opa = (cats == 1) & m
    const_slot = np.where(is_lit, litsc, NI)  # 0..9 onehot, 10 zeros

    # Worklists W[p]: (row, node) updates needed at pass p.
    Wr = [None] * (NP + 1)
    Wn = [None] * (NP + 1)
    r10 = np.nonzero(cats[:, 0] == 1)[0].astype(np.int64)
    Wr[NP], Wn[NP] = r10, np.zeros(len(r10), np.int64)
    need = np.zeros((B, N), bool)
    for p in range(NP, 1, -1):
        r, n = Wr[p], Wn[p]
        cr = np.concatenate([r, r])
        cn = np.concatenate([left[r, n], right[r, n]])
        keep = is_opa[cr, cn]
        need[:] = False
        need[cr[keep], cn[keep]] = True
        rr, nn = np.nonzero(need)
        Wr[p - 1], Wn[p - 1] = rr.astype(np.int64), nn.astype(np.int64)

    # Bin rows into NG groups per core, balancing total updates per group.
    weight = np.zeros(B, np.int64)
    for p in range(1, NP + 1):
        np.add.at(weight, Wr[p], 1)
    G = np.zeros(B, np.int64)
    for c in range(NCORES):
        rows = np.arange(c * RPC, (c + 1) * RPC)
        order = rows[np.argsort(-weight[rows], kind="stable")]
        load = np.zeros(NG, np.int64)
        for rr_ in order:
            g = int(load.argmin())
            G[rr_] = g
            load[g] += weight[rr_]

    # Per-pass group-local ids and padded per-group size Q_p.
    Qp = np.zeros(NP + 1, np.int64)
    gid = [None] * (NP + 1)
    for p in range(1, NP + 1):
        r = Wr[p]
        core = r // RPC
        grp = G[r]
        key = core * NG + grp
        order = np.argsort(key, kind="stable")
        ks = key[order]
        u = np.arange(len(ks), dtype=np.int64)
        if len(ks):
            first = np.r_[True, ks[1:] != ks[:-1]]
            seg_idx = np.nonzero(first)[0]
            u = u - seg_idx[np.cumsum(first) - 1]
        ul = np.empty(len(ks), np.int64)
        ul[order] = u
        cnt = np.bincount(key, minlength=NCORES * NG) if len(r) else np.zeros(NCORES * NG, np.int64)
        mx = int(cnt.max()) if len(r) else 0
        Qp[p] = max(4, mx)
        gid[p] = (core, grp, ul)
    assert int(max(Qp[1:])) <= QLIM, f"group too large: {Qp}"
    assert 2 * NG * int(max(Qp[1:])) <= 2 * 448, f"PSUM chunking unsupported: {Qp}"

    # Sel (one-hot row-select) and M30 (op-variant mask) per pass 2..NP,
    # plus the host-built pass-1 lr30m.
    sel_blocks = []   # each (128, 2*NG*Qp) bf16, cols (g, h, u)
    m30_blocks = []   # each (30, 2*NG*Qp) bf16, cols (h, g, u)
    lr1 = None
    slot_prev = np.full((B, N), -1, np.int64)
    eyeext = np.concatenate([np.eye(NI, dtype=np.float32),
                             np.zeros((NI, 1), np.float32)], axis=1)  # (10, 11)
    for p in range(1, NP + 1):
        r, n = Wr[p], Wn[p]
        core, grp, ul = gid[p]
        Q = int(Qp[p])
        PT = NG * Q
        if p == 1:
            # children are all constants; build lr30m directly (30, 2*PT)
            lr = np.zeros((NCORES, 30, 2 * PT), np.float32)
            for h, ch in ((0, left[r, n]), (1, right[r, n])):
                cs = const_slot[r, ch]       # 0..10
                val = eyeext[:, cs]          # (10, len)
                col = h * PT + grp * Q + ul
                # mask o=0 only
                lr[core, 0:NI, col] = val.T
            lr1 = lr
        else:
            sel = np.zeros((NCORES, 128, 2 * PT), np.float32)
            m30 = np.zeros((NCORES, 30, 2 * PT), np.float32)
            for h, ch in ((0, left[r, n]), (1, right[r, n])):
                isop = is_opa[r, ch]
                rowi = np.where(isop, slot_prev[r, ch], CROW + const_slot[r, ch])
                ovar = np.where(isop, opsc[r, ch], 0)
                scol = (2 * grp + h) * Q + ul        # sel cols (g, h, u)
                mcol = h * PT + grp * Q + ul         # m30 cols (h, g, u)
                sel[core, rowi, scol] = 1.0
                for o in range(NO):
                    pick = ovar == o
                    m30[core[pick], o * NI : (o + 1) * NI, mcol[pick]] = 1.0
            sel_blocks.append(sel)
            m30_blocks.append(m30)
        if p < NP:
            slot_prev = np.full((B, N), -1, np.int64)
            slot_prev[r, n] = ul

    sel_all = np.concatenate(sel_blocks, axis=2)  # (NCORES, 128, SelCols)
    m30_all = np.concatenate(m30_blocks, axis=2)  # (NCORES, 30, SelCols)

    return dict(
        Qp=Qp, sel=sel_all, m30=m30_all, lr1=lr1,
        r10=r10, gid10=gid[NP], opsc=opsc, litsc=litsc, is_lit=is_lit,
    )


def _build_nc(Qp, SelCols):
    Q10 = int(Qp[NP])
    Qmax = int(max(Qp[1:]))
    PTmax = NG * Qmax
    W30 = 30 * NG  # stT3 / zT free width
    nc = bacc.Bacc(None)
    consts30 = nc.dram_tensor("consts30", [NCONST_ROWS, W30], BF, kind="ExternalInput")
    wmat30 = nc.dram_tensor("wmat30", [100, 30], BF, kind="ExternalInput")
    repl30 = nc.dram_tensor("repl30", [30, 100], BF, kind="ExternalInput")
    reprm30 = nc.dram_tensor("reprm30", [30, 100], BF, kind="ExternalInput")
    sel_in = nc.dram_tensor("sel", [128, SelCols], BF, kind="ExternalInput")
    m30_in = nc.dram_tensor("m30", [30, SelCols], BF, kind="ExternalInput")
    PT1 = NG * int(Qp[1])
    lr1_in = nc.dram_tensor("lr1", [30, 2 * PT1], BF, kind="ExternalInput")
    outz = nc.dram_tensor("outz", [Q10, W30], F32, kind="ExternalOutput")

    with ExitStack() as ctx:
        tc = ctx.enter_context(tile.TileContext(nc))
        singles = ctx.enter_context(tc.tile_pool(name="singles", bufs=1))
        psum = ctx.enter_context(tc.tile_pool(name="psum", bufs=1, space="PSUM"))

        # constants -- pass-1 critical ones first, split across both DGE rings
        repl_sb = singles.tile([30, 100], BF)
        nc.scalar.dma_start(out=repl_sb[:, :], in_=repl30[:, :])
        lr1_sb = singles.tile([30, 2 * PT1], BF)
        nc.sync.dma_start(out=lr1_sb[:, :], in_=lr1_in[:, :])
        reprm_sb = singles.tile([30, 100], BF)
        nc.scalar.dma_start(out=reprm_sb[:, :], in_=reprm30[:, :])
        w_sb = singles.tile([100, 30], BF)
        nc.sync.dma_start(out=w_sb[:, :], in_=wmat30[:, :])
        stT3 = singles.tile([128, W30], BF)
        nc.vector.memset(stT3[:, :], 0.0)
        nc.scalar.dma_start(out=stT3[CROW : CROW + NCONST_ROWS, :], in_=consts30[:, :])
        sel_sb = singles.tile([128, SelCols], BF)
        nc.sync.dma_start(out=sel_sb[:, :], in_=sel_in[:, :])
        m30_sb = singles.tile([30, SelCols], BF)
        nc.scalar.dma_start(out=m30_sb[:, :], in_=m30_in[:, :])

        # work tiles (max size, sliced per pass)
        lr30m = singles.tile([30, 2 * PTmax], BF)
        lsb = singles.tile([100, PTmax], F32)
        outer = singles.tile([100, PTmax], BF)
        esb = singles.tile([Qmax, W30], F32)
        zs = singles.tile([Qmax, NO * NG], F32)
        rz = singles.tile([Qmax, NO * NG], F32)
        zout = singles.tile([Q10, W30], F32)
        lr30a = psum.tile([30, 2 * 2 * Qmax], F32, tag="lr30a")  # groups 0,1
        lr30b = psum.tile([30, 2 * 2 * Qmax], F32, tag="lr30b")  # groups 2,3
        ps_l = psum.tile([100, PTmax], F32, tag="ps_l")
        ps_r = psum.tile([100, PTmax], F32, tag="ps_r")
        zT = psum.tile([128, W30], F32, tag="zT")

        soff = 0
        for p in range(1, NP + 1):
            Q = int(Qp[p])
            PT = NG * Q
            if p == 1:
                lrm = lr1_sb
            else:
                lrm = lr30m[:, 0 : 2 * PT]
                halves = (lr30a, lr30b)
                for g in range(NG):
                    ph = halves[g // 2]
                    pslice = ph[:, (g % 2) * 2 * Q : (g % 2 + 1) * 2 * Q]
                    nc.tensor.matmul(
                        pslice,
                        stT3[:, 30 * g : 30 * (g + 1)],
                        sel_sb[:, soff + 2 * g * Q : soff + 2 * (g + 1) * Q],
                        start=True, stop=True,
                    )
                    # mask-mul: lr30m[:, h*PT + g*Q + u] = lr30[(g,h,u)] * m30
                    dst = lrm.rearrange("m (h gg u) -> m h gg u", h=2, gg=NG)[:, :, g, :]
                    msrc = m30_sb[:, soff : soff + 2 * PT].rearrange(
                        "m (h gg u) -> m h gg u", h=2, gg=NG)[:, :, g, :]
                    nc.vector.tensor_mul(
                        dst,
                        pslice.rearrange("m (h u) -> m h u", h=2),
                        msrc,
                    )
                soff += 2 * PT
            nc.tensor.matmul(ps_l[:, 0:PT], repl_sb[:, :], lrm[:, 0:PT],
                             start=True, stop=True)
            nc.scalar.copy(lsb[:, 0:PT], ps_l[:, 0:PT])
            nc.tensor.matmul(ps_r[:, 0:PT], reprm_sb[:, :], lrm[:, PT : 2 * PT],
                             start=True, stop=True)
            nc.vector.tensor_mul(outer[:, 0:PT], lsb[:, 0:PT], ps_r[:, 0:PT])
            for g in range(NG):
                nc.tensor.matmul(
                    zT[0:Q, 30 * g : 30 * (g + 1)],
                    outer[:, g * Q : (g + 1) * Q],
                    w_sb[:, :],
                    start=True, stop=True,
                )
            if p == NP:
                nc.scalar.copy(zout[:, :], zT[0:Q, :])
                nc.sync.dma_start(out=outz[:, :], in_=zout[:, :])
                continue
            nc.scalar.activation(esb[0:Q, :], zT[0:Q, :],
                                 mybir.ActivationFunctionType.Exp)
            nc.vector.tensor_reduce(
                zs[0:Q, :],
                esb[0:Q, :].rearrange("q (t k) -> q t k", k=NI),
                axis=mybir.AxisListType.X, op=mybir.AluOpType.add,
            )
            nc.vector.reciprocal_approx_fast(out=rz[0:Q, :], in_=zs[0:Q, :])
            nc.vector.tensor_mul(
                stT3[0:Q, :].rearrange("q (t k) -> q t k", k=NI),
                esb[0:Q, :].rearrange("q (t k) -> q t k", k=NI),
                rz[0:Q, :].rearrange("q (t o) -> q t o", o=1).broadcast_to(
                    [Q, NO * NG, NI]),
            )
    nc.finalize()
    return nc


NCONST_ROWS = NI + 1  # 11 const rows (one-hots + zero vector)


def kernel(op_table, cats, ops, lits, left, right, mask):
    global LAST_RESULTS
    op_table = np.asarray(op_table, np.float32)
    plan = _plan(np.asarray(cats), np.asarray(ops), np.asarray(lits),
                 np.asarray(left), np.asarray(right), np.asarray(mask))
    Qp = plan["Qp"]
    SelCols = plan["sel"].shape[2]

    nc = _build_nc(Qp, SelCols)

    # host-side constants
    eyeext = np.concatenate([np.eye(NI, dtype=np.float32),
                             np.zeros((NI, 1), np.float32)], axis=1)
    consts30 = np.zeros((NCONST_ROWS, 30 * NG), np.float32)
    for s in range(NCONST_ROWS):
        vec = eyeext[:, s]  # (10,)
        consts30[s, :] = np.tile(vec, 3 * NG)
    w30 = op_table.transpose(1, 2, 0, 3).reshape(100, 30)
    repl = np.kron(np.eye(NI), np.ones((1, NI))).astype(np.float32)
    reprm = np.tile(np.eye(NI), (1, NI)).astype(np.float32)
    repl30 = np.tile(repl, (3, 1))    # (30, 100)
    reprm30 = np.tile(reprm, (3, 1))  # (30, 100)

    bf = ml_dtypes.bfloat16
    in_maps = []
    for c in range(NCORES):
        in_maps.append({
            "consts30": consts30.astype(bf),
            "wmat30": w30.astype(bf),
            "repl30": repl30.astype(bf), "reprm30": reprm30.astype(bf),
            "sel": np.ascontiguousarray(plan["sel"][c].astype(bf)),
            "m30": np.ascontiguousarray(plan["m30"][c].astype(bf)),
            "lr1": np.ascontiguousarray(plan["lr1"][c].astype(bf)),
        })

    res = run_bass_kernel_spmd(nc, in_maps, list(range(NCORES)), trace=TRACE)
    LAST_RESULTS = res

    # Assemble the full (B, NI) output on the host (index selection only).
    out = np.zeros((B, NI), np.float32)
    litsc, is_lit = plan["litsc"], plan["is_lit"]
    lit_rows = np.nonzero(cats[:, 0] == 0)[0]
    lr_active = is_lit[lit_rows, 0]
    oh = 10.0 * np.eye(NI, dtype=np.float32)[litsc[lit_rows, 0]]
    out[lit_rows] = np.where(lr_active[:, None], oh, 0.0)

    r10, opsc = plan["r10"], plan["opsc"]
    core10, grp10, ul10 = plan["gid10"]
    for c in range(NCORES):
        z = np.asarray(res.results[c]["outz"])  # (Q10, 30*NG)
        selmask = core10 == c
        rows = r10[selmask]
        u = ul10[selmask]
        g = grp10[selmask]
        o = opsc[rows, 0]
        col0 = 30 * g + NI * o
        for k in range(NI):
            out[rows, k] = z[u, col0 + k]
    return out


# revision 8
# speedup vs baseline: 4.7444x; 1.0032x over previous
"""Trainium2 Bass kernel for nn_CruxMiniCircuit (gnn_message_passing).

Reference semantics: B independent rows; each row is a circuit of N nodes
(literal nodes hold a fixed one-hot distribution over 10 ints, op nodes
combine left/right child distributions through a per-op bilinear table
followed by softmax).  The reference runs 10 synchronous passes over all
nodes and returns only the root (node 0) logits per row.

Key observation: the output depends only on node 0's dependency cone
unrolled 10 passes deep.  Literal children are compile-time constants
(one-hot vectors) and op nodes at pass 0 are zero, so the per-row
worklists are tiny (mean ~5 updates/row for the benchmark distribution).
The host precomputes integer worklists / gather indices; the device does
all floating-point math.

v3 device pipeline ("all-matmul", zero GPSIMD): the per-pass state is
kept TRANSPOSED in SBUF as stT3[slot, 30*NG] (slots on partitions, all
three op-variant softmaxes on the free dim), so the child-value gather
becomes 4 small selection matmuls against constant one-hot matrices
(host-built, DMA'd at startup) and the op-variant choice becomes one
elementwise multiply with a host-built 0/1 mask.  The bilinear logits
are produced TRANSPOSED (zT[item, 30]) by using the outer-product tile
as the matmul stationary operand, which lets softmax run along the free
dim: exp -> grouped free-dim reduce -> fast-approx reciprocal ->
stride-0 broadcast multiply.  No ap_gather (whose ~2.7us/invocation
hidden dispatch stall dominated v2), no scatter, no value buffer.
Everything is bf16 matmuls (single-pump PE) + f32 softmax.

Sharding: pure data parallel over the batch dim (B=2048 -> 256 rows on
each of the 8 NeuronCores), per the sharding hint.  No collectives are
needed for the forward pass.
"""

import sys
from contextlib import ExitStack

import numpy as np

sys.path.insert(0, "/opt/trn_rl_repo")

import ml_dtypes

import concourse.bass as bass
import concourse.tile as tile
from concourse import bacc, mybir
from concourse.bass_utils import run_bass_kernel_spmd

B, N = 2048, 1023
NI, NO, NP = 10, 3, 10  # n_ints, n_ops, n_passes
NCORES = 8
RPC = B // NCORES  # rows per core
NG = 4  # groups per core: column blocks of stT3 / outer / zT
CROW = 117  # stT3 row where the 11 constant one-hot slots start
QLIM = CROW  # value slots per group must fit below the const rows

TRACE = False
LAST_RESULTS = None

BF = mybir.dt.bfloat16
F32 = mybir.dt.float32


def _plan(cats, ops, lits, left, right, mask):
    """Integer-only preprocessing: worklists, binning, Sel/M30 matrices."""
    left = np.clip(left.astype(np.int64), 0, N - 1)
    right = np.clip(right.astype(np.int64), 0, N - 1)
    opsc = np.clip(ops.astype(np.int64), 0, NO - 1)
    litsc = np.clip(lits.astype(np.int64), 0, NI - 1)
    m = mask.astype(bool)
    is_lit = (cats == 0) & m
    is_opa = (cats == 1) & m
    const_slot = np.where(is_lit, litsc, NI)  # 0..9 onehot, 10 zeros

    # Worklists W[p]: (row, node) updates needed at pass p.
    Wr = [None] * (NP + 1)
    Wn = [None] * (NP + 1)
    r10 = np.nonzero(cats[:, 0] == 1)[0].astype(np.int64)
    Wr[NP], Wn[NP] = r10, np.zeros(len(r10), np.int64)
    need = np.zeros((B, N), bool)
    for p in range(NP, 1, -1):
        r, n = Wr[p], Wn[p]
        cr = np.concatenate([r, r])
        cn = np.concatenate([left[r, n], right[r, n]])
        keep = is_opa[cr, cn]
        need[:] = False
        need[cr[keep], cn[keep]] = True
        rr, nn = np.nonzero(need)
        Wr[p - 1], Wn[p - 1] = rr.astype(np.int64), nn.astype(np.int64)

    # Bin rows into NG groups per core, balancing total updates per group.
    weight = np.zeros(B, np.int64)
    for p in range(1, NP + 1):
        np.add.at(weight, Wr[p], 1)
    G = np.zeros(B, np.int64)
    for c in range(NCORES):
        rows = np.arange(c * RPC, (c + 1) * RPC)
        order = rows[np.argsort(-weight[rows], kind="stable")]
        load = np.zeros(NG, np.int64)
        for rr_ in order:
            g = int(load.argmin())
            G[rr_] = g
            load[g] += weight[rr_]

    # Per-pass group-local ids and padded per-group size Q_p.
    Qp = np.zeros(NP + 1, np.int64)
    gid = [None] * (NP + 1)
    for p in range(1, NP + 1):
        r = Wr[p]
        core = r // RPC
        grp = G[r]
        key = core * NG + grp
        order = np.argsort(key, kind="stable")
        ks = key[order]
        u = np.arange(len(ks), dtype=np.int64)
        if len(ks):
            first = np.r_[True, ks[1:] != ks[:-1]]
            seg_idx = np.nonzero(first)[0]
            u = u - seg_idx[np.cumsum(first) - 1]
        ul = np.empty(len(ks), np.int64)
        ul[order] = u
        cnt = np.bincount(key, minlength=NCORES * NG) if len(r) else np.zeros(NCORES * NG, np.int64)
        mx = int(cnt.max()) if len(r) else 0
        Qp[p] = max(4, mx)
        gid[p] = (core, grp, ul)
    assert int(max(Qp[1:])) <= QLIM, f"group too large: {Qp}"
    assert 2 * NG * int(max(Qp[1:])) <= 2 * 448, f"PSUM chunking unsupported: {Qp}"

    # Sel (one-hot row-select) and M30 (op-variant mask) per pass 2..NP,
    # plus the host-built pass-1 lr30m.
    sel_blocks = []   # each (128, 2*NG*Qp) bf16, cols (g, h, u)
    m30_blocks = []   # each (30, 2*NG*Qp) bf16, cols (h, g, u)
    lr1 = None
    slot_prev = np.full((B, N), -1, np.int64)
    eyeext = np.concatenate([np.eye(NI, dtype=np.float32),
                             np.zeros((NI, 1), np.float32)], axis=1)  # (10, 11)
    for p in range(1, NP + 1):
        r, n = Wr[p], Wn[p]
        core, grp, ul = gid[p]
        Q = int(Qp[p])
        PT = NG * Q
        if p == 1:
            # children are all constants; build lr30m directly (30, 2*PT)
            lr = np.zeros((NCORES, 30, 2 * PT), np.float32)
            for h, ch in ((0, left[r, n]), (1, right[r, n])):
                cs = const_slot[r, ch]       # 0..10
                val = eyeext[:, cs]          # (10, len)
                col = h * PT + grp * Q + ul
                # mask o=0 only
                lr[core, 0:NI, col] = val.T
            lr1 = lr
        else:
            sel = np.zeros((NCORES, 128, 2 * PT), np.float32)
            m30 = np.zeros((NCORES, 30, 2 * PT), np.float32)
            for h, ch in ((0, left[r, n]), (1, right[r, n])):
                isop = is_opa[r, ch]
                rowi = np.where(isop, slot_prev[r, ch], CROW + const_slot[r, ch])
                ovar = np.where(isop, opsc[r, ch], 0)
                scol = (2 * grp + h) * Q + ul        # sel cols (g, h, u)
                mcol = h * PT + grp * Q + ul         # m30 cols (h, g, u)
                sel[core, rowi, scol] = 1.0
                for o in range(NO):
                    pick = ovar == o
                    m30[core[pick], o * NI : (o + 1) * NI, mcol[pick]] = 1.0
            sel_blocks.append(sel)
            m30_blocks.append(m30)
        if p < NP:
            slot_prev = np.full((B, N), -1, np.int64)
            slot_prev[r, n] = ul

    sel_all = np.concatenate(sel_blocks, axis=2)  # (NCORES, 128, SelCols)
    m30_all = np.concatenate(m30_blocks, axis=2)  # (NCORES, 30, SelCols)

    return dict(
        Qp=Qp, sel=sel_all, m30=m30_all, lr1=lr1,
        r10=r10, gid10=gid[NP], opsc=opsc, litsc=litsc, is_lit=is_lit,
    )


def _build_nc(Qp, SelCols):
    Q10 = int(Qp[NP])
    Qmax = int(max(Qp[1:]))
    PTmax = NG * Qmax
    W30 = 30 * NG  # stT3 / zT free width
    nc = bacc.Bacc(None)
    consts30 = nc.dram_tensor("consts30", [NCONST_ROWS, W30], BF, kind="ExternalInput")
    wmat30 = nc.dram_tensor("wmat30", [100, 30], BF, kind="ExternalInput")
    replpair = nc.dram_tensor("replpair", [30, 200], BF, kind="ExternalInput")
    sel_in = nc.dram_tensor("sel", [128, SelCols], BF, kind="ExternalInput")
    m30_in = nc.dram_tensor("m30", [30, SelCols], BF, kind="ExternalInput")
    PT1 = NG * int(Qp[1])
    lr1_in = nc.dram_tensor("lr1", [30, 2 * PT1], BF, kind="ExternalInput")
    outz = nc.dram_tensor("outz", [Q10, W30], F32, kind="ExternalOutput")

    with ExitStack() as ctx:
        tc = ctx.enter_context(tile.TileContext(nc))
        singles = ctx.enter_context(tc.tile_pool(name="singles", bufs=1))
        psum = ctx.enter_context(tc.tile_pool(name="psum", bufs=1, space="PSUM"))

        # constants -- pass-1 critical ones first, split across both DGE rings
        replp_sb = singles.tile([30, 200], BF)
        nc.scalar.dma_start(out=replp_sb[:, :], in_=replpair[:, :])
        repl_sb = replp_sb[:, 0:100]
        reprm_sb = replp_sb[:, 100:200]
        lr1_sb = singles.tile([30, 2 * PT1], BF)
        nc.sync.dma_start(out=lr1_sb[:, :], in_=lr1_in[:, :])
        w_sb = singles.tile([100, 30], BF)
        nc.sync.dma_start(out=w_sb[:, :], in_=wmat30[:, :])
        stT3 = singles.tile([128, W30], BF)
        nc.vector.memset(stT3[:, :], 0.0)
        nc.scalar.dma_start(out=stT3[CROW : CROW + NCONST_ROWS, :], in_=consts30[:, :])
        sel_sb = singles.tile([128, SelCols], BF)
        nc.sync.dma_start(out=sel_sb[:, :], in_=sel_in[:, :])
        m30_sb = singles.tile([30, SelCols], BF)
        nc.scalar.dma_start(out=m30_sb[:, :], in_=m30_in[:, :])

        # work tiles (max size, sliced per pass)
        lr30m = singles.tile([30, 2 * PTmax], BF)
        lsb = singles.tile([100, PTmax], F32)
        outer = singles.tile([100, PTmax], BF)
        esb = singles.tile([Qmax, W30], F32)
        zs = singles.tile([Qmax, NO * NG], F32)
        rz = singles.tile([Qmax, NO * NG], F32)
        zout = singles.tile([Q10, W30], F32)
        lr30a = psum.tile([30, 2 * 2 * Qmax], F32, tag="lr30a")  # groups 0,1
        lr30b = psum.tile([30, 2 * 2 * Qmax], F32, tag="lr30b")  # groups 2,3
        ps_l = psum.tile([100, PTmax], F32, tag="ps_l")
        ps_r = psum.tile([100, PTmax], F32, tag="ps_r")
        zT = psum.tile([128, W30], F32, tag="zT")

        soff = 0
        for p in range(1, NP + 1):
            Q = int(Qp[p])
            PT = NG * Q
            if p == 1:
                lrm = lr1_sb
            else:
                lrm = lr30m[:, 0 : 2 * PT]
                halves = (lr30a, lr30b)
                for g in range(NG):
                    ph = halves[g // 2]
                    pslice = ph[:, (g % 2) * 2 * Q : (g % 2 + 1) * 2 * Q]
                    nc.tensor.matmul(
                        pslice,
                        stT3[:, 30 * g : 30 * (g + 1)],
                        sel_sb[:, soff + 2 * g * Q : soff + 2 * (g + 1) * Q],
                        start=True, stop=True,
                    )
                    # mask-mul: lr30m[:, h*PT + g*Q + u] = lr30[(g,h,u)] * m30
                    dst = lrm.rearrange("m (h gg u) -> m h gg u", h=2, gg=NG)[:, :, g, :]
                    msrc = m30_sb[:, soff : soff + 2 * PT].rearrange(
                        "m (h gg u) -> m h gg u", h=2, gg=NG)[:, :, g, :]
                    nc.vector.tensor_mul(
                        dst,
                        pslice.rearrange("m (h u) -> m h u", h=2),
                        msrc,
                    )
                soff += 2 * PT
            nc.tensor.matmul(ps_l[:, 0:PT], repl_sb, lrm[:, 0:PT],
                             start=True, stop=True)
            nc.scalar.copy(lsb[:, 0:PT], ps_l[:, 0:PT])
            nc.tensor.matmul(ps_r[:, 0:PT], reprm_sb, lrm[:, PT : 2 * PT],
                             start=True, stop=True)
            nc.vector.tensor_mul(outer[:, 0:PT], lsb[:, 0:PT], ps_r[:, 0:PT])
            for g in range(NG):
                nc.tensor.matmul(
                    zT[0:Q, 30 * g : 30 * (g + 1)],
                    outer[:, g * Q : (g + 1) * Q],
                    w_sb[:, :],
                    start=True, stop=True,
                )
            if p == NP:
                nc.scalar.copy(zout[:, :], zT[0:Q, :])
                nc.sync.dma_start(out=outz[:, :], in_=zout[:, :])
                continue
            nc.scalar.activation(esb[0:Q, :], zT[0:Q, :],
                                 mybir.ActivationFunctionType.Exp)
            nc.vector.tensor_reduce(
                zs[0:Q, :],
                esb[0:Q, :].rearrange("q (t k) -> q t k", k=NI),
                axis=mybir.AxisListType.X, op=mybir.AluOpType.add,
            )
            nc.vector.reciprocal_approx_fast(out=rz[0:Q, :], in_=zs[0:Q, :])
            nc.vector.tensor_mul(
                stT3[0:Q, :].rearrange("q (t k) -> q t k", k=NI),
                esb[0:Q, :].rearrange("q (t k) -> q t k", k=NI),
                rz[0:Q, :].rearrange("q (t o) -> q t o", o=1).broadcast_to(
                    [Q, NO * NG, NI]),
            )
    nc.finalize()
    return nc


NCONST_ROWS = NI + 1  # 11 const rows (one-hots + zero vector)


def kernel(op_table, cats, ops, lits, left, right, mask):
    global LAST_RESULTS
    op_table = np.asarray(op_table, np.float32)
    plan = _plan(np.asarray(cats), np.asarray(ops), np.asarray(lits),
                 np.asarray(left), np.asarray(right), np.asarray(mask))
    Qp = plan["Qp"]
    SelCols = plan["sel"].shape[2]

    nc = _build_nc(Qp, SelCols)

    # host-side constants
    eyeext = np.concatenate([np.eye(NI, dtype=np.float32),
                             np.zeros((NI, 1), np.float32)], axis=1)
    consts30 = np.zeros((NCONST_ROWS, 30 * NG), np.float32)
    for s in range(NCONST_ROWS):
        vec = eyeext[:, s]  # (10,)
        consts30[s, :] = np.tile(vec, 3 * NG)
    w30 = op_table.transpose(1, 2, 0, 3).reshape(100, 30)
    repl = np.kron(np.eye(NI), np.ones((1, NI))).astype(np.float32)
    reprm = np.tile(np.eye(NI), (1, NI)).astype(np.float32)
    repl30 = np.tile(repl, (3, 1))    # (30, 100)
    reprm30 = np.tile(reprm, (3, 1))  # (30, 100)

    bf = ml_dtypes.bfloat16
    in_maps = []
    for c in range(NCORES):
        in_maps.append({
            "consts30": consts30.astype(bf),
            "wmat30": w30.astype(bf),
            "replpair": np.concatenate([repl30, reprm30], axis=1).astype(bf),
            "sel": np.ascontiguousarray(plan["sel"][c].astype(bf)),
            "m30": np.ascontiguousarray(plan["m30"][c].astype(bf)),
            "lr1": np.ascontiguousarray(plan["lr1"][c].astype(bf)),
        })

    res = run_bass_kernel_spmd(nc, in_maps, list(range(NCORES)), trace=TRACE)
    LAST_RESULTS = res

    # Assemble the full (B, NI) output on the host (index selection only).
    out = np.zeros((B, NI), np.float32)
    litsc, is_lit = plan["litsc"], plan["is_lit"]
    lit_rows = np.nonzero(cats[:, 0] == 0)[0]
    lr_active = is_lit[lit_rows, 0]
    oh = 10.0 * np.eye(NI, dtype=np.float32)[litsc[lit_rows, 0]]
    out[lit_rows] = np.where(lr_active[:, None], oh, 0.0)

    r10, opsc = plan["r10"], plan["opsc"]
    core10, grp10, ul10 = plan["gid10"]
    for c in range(NCORES):
        z = np.asarray(res.results[c]["outz"])  # (Q10, 30*NG)
        selmask = core10 == c
        rows = r10[selmask]
        u = ul10[selmask]
        g = grp10[selmask]
        o = opsc[rows, 0]
        col0 = 30 * g + NI * o
        for k in range(NI):
            out[rows, k] = z[u, col0 + k]
    return out


# revision 10
# speedup vs baseline: 5.2461x; 1.1057x over previous
"""Trainium2 Bass kernel for nn_CruxMiniCircuit (gnn_message_passing).

Reference semantics: B independent rows; each row is a circuit of N nodes
(literal nodes hold a fixed one-hot distribution over 10 ints, op nodes
combine left/right child distributions through a per-op bilinear table
followed by softmax).  The reference runs 10 synchronous passes over all
nodes and returns only the root (node 0) logits per row.

Key observation: the output depends only on node 0's dependency cone
unrolled 10 passes deep.  Literal children are compile-time constants
(one-hot vectors) and op nodes at pass 0 are zero, so the per-row
worklists are tiny (mean ~5 updates/row for the benchmark distribution).
The host precomputes integer worklists / gather indices; the device does
all floating-point math.

v4 device pipeline ("all-matmul", zero GPSIMD, two pipelined streams):
the per-pass state is kept TRANSPOSED in SBUF as stT[slot, 30*G] (slots
on partitions, all three op-variant softmaxes on the free dim), so the
child-value gather becomes small selection matmuls against constant
one-hot matrices (host-built, DMA'd at startup) and the op-variant
choice becomes an elementwise multiply with a host-built 0/1 mask.  The
bilinear logits are produced TRANSPOSED (zT[item, 30]) by using the
outer-product tile as the matmul stationary operand, which lets softmax
run along the free dim: exp -> grouped free-dim reduce -> fast-approx
reciprocal -> stride-0 broadcast multiply.  Rows are binned into 4
groups split over 2 fully independent streams whose ops are interleaved
phase-by-phase, so each engine's in-order queue alternates streams and
one stream's compute hides the other's cross-engine latency.  No
ap_gather (whose ~2.7us/invocation hidden dispatch stall dominated v2),
no scatter, no value buffer.  All matmuls bf16 (single-pump PE),
softmax in f32.

Sharding: pure data parallel over the batch dim (B=2048 -> 256 rows on
each of the 8 NeuronCores), per the sharding hint.  No collectives are
needed for the forward pass.
"""

import sys
from contextlib import ExitStack

import numpy as np

sys.path.insert(0, "/opt/trn_rl_repo")

import ml_dtypes

import concourse.bass as bass
import concourse.tile as tile
from concourse import bacc, mybir
from concourse.bass_utils import run_bass_kernel_spmd

B, N = 2048, 1023
NI, NO, NP = 10, 3, 10  # n_ints, n_ops, n_passes
NCORES = 8
RPC = B // NCORES  # rows per core
NS = 2   # independent pipelined streams
GPS = 2  # groups per stream
NG = NS * GPS  # total groups per core
CROW = 117  # stT row where the 11 constant one-hot slots start
QLIM = CROW
NCONST_ROWS = NI + 1

TRACE = False
LAST_RESULTS = None

BF = mybir.dt.bfloat16
F32 = mybir.dt.float32


def _plan(cats, ops, lits, left, right, mask):
    """Integer-only preprocessing: worklists, binning, Sel/M30 matrices."""
    left = np.clip(left.astype(np.int64), 0, N - 1)
    right = np.clip(right.astype(np.int64), 0, N - 1)
    opsc = np.clip(ops.astype(np.int64), 0, NO - 1)
    litsc = np.clip(lits.astype(np.int64), 0, NI - 1)
    m = mask.astype(bool)
    is_lit = (cats == 0) & m
    is_opa = (cats == 1) & m
    const_slot = np.where(is_lit, litsc, NI)  # 0..9 onehot, 10 zeros

    # Worklists W[p]: (row, node) updates needed at pass p.
    Wr = [None] * (NP + 1)
    Wn = [None] * (NP + 1)
    r10 = np.nonzero(cats[:, 0] == 1)[0].astype(np.int64)
    Wr[NP], Wn[NP] = r10, np.zeros(len(r10), np.int64)
    need = np.zeros((B, N), bool)
    for p in range(NP, 1, -1):
        r, n = Wr[p], Wn[p]
        cr = np.concatenate([r, r])
        cn = np.concatenate([left[r, n], right[r, n]])
        keep = is_opa[cr, cn]
        need[:] = False
        need[cr[keep], cn[keep]] = True
        rr, nn = np.nonzero(need)
        Wr[p - 1], Wn[p - 1] = rr.astype(np.int64), nn.astype(np.int64)

    # Bin rows into NG groups per core, balancing total updates per group.
    weight = np.zeros(B, np.int64)
    for p in range(1, NP + 1):
        np.add.at(weight, Wr[p], 1)
    G = np.zeros(B, np.int64)
    for c in range(NCORES):
        rows = np.arange(c * RPC, (c + 1) * RPC)
        order = rows[np.argsort(-weight[rows], kind="stable")]
        load = np.zeros(NG, np.int64)
        for rr_ in order:
            g = int(load.argmin())
            G[rr_] = g
            load[g] += weight[rr_]

    # Per-pass group-local ids and padded per-group size Q_p.
    Qp = np.zeros(NP + 1, np.int64)
    gid = [None] * (NP + 1)
    for p in range(1, NP + 1):
        r = Wr[p]
        core = r // RPC
        grp = G[r]
        key = core * NG + grp
        order = np.argsort(key, kind="stable")
        ks = key[order]
        u = np.arange(len(ks), dtype=np.int64)
        if len(ks):
            first = np.r_[True, ks[1:] != ks[:-1]]
            seg_idx = np.nonzero(first)[0]
            u = u - seg_idx[np.cumsum(first) - 1]
        ul = np.empty(len(ks), np.int64)
        ul[order] = u
        cnt = np.bincount(key, minlength=NCORES * NG) if len(r) else np.zeros(NCORES * NG, np.int64)
        mx = int(cnt.max()) if len(r) else 0
        Qp[p] = max(4, mx)
        gid[p] = (core, grp, ul)
    assert int(max(Qp[1:])) <= QLIM, f"group too large: {Qp}"
    assert GPS * int(max(Qp[1:])) <= 448, f"PSUM chunking unsupported: {Qp}"

    # Sel (one-hot row-select) and M30 (op-variant mask) per pass 2..NP,
    # plus the host-built pass-1 lr30m.  Column layouts per pass:
    #   sel: (s, j, h, u)  -> stream s slice [s*4Q : (s+1)*4Q]
    #   m30: (s, h, j, u)  -> stream s slice [s*2*PTs : ...], PTs = GPS*Q
    sel_blocks = []
    m30_blocks = []
    lr1 = None
    slot_prev = np.full((B, N), -1, np.int64)
    eyeext = np.concatenate([np.eye(NI, dtype=np.float32),
                             np.zeros((NI, 1), np.float32)], axis=1)  # (10, 11)
    for p in range(1, NP + 1):
        r, n = Wr[p], Wn[p]
        core, grp, ul = gid[p]
        strm = grp // GPS
        j = grp % GPS
        Q = int(Qp[p])
        PTs = GPS * Q
        if p == 1:
            lr = np.zeros((NCORES, 30, NS * 2 * PTs), np.float32)
            for h, ch in ((0, left[r, n]), (1, right[r, n])):
                cs = const_slot[r, ch]
                val = eyeext[:, cs]  # (10, len)
                col = strm * 2 * PTs + h * PTs + j * Q + ul
                lr[core, 0:NI, col] = val.T
            lr1 = lr
        else:
            sel = np.zeros((NCORES, 128, NS * 2 * PTs), np.float32)
            m30 = np.zeros((NCORES, 30, NS * 2 * PTs), np.float32)
            for h, ch in ((0, left[r, n]), (1, right[r, n])):
                isop = is_opa[r, ch]
                rowi = np.where(isop, slot_prev[r, ch], CROW + const_slot[r, ch])
                ovar = np.where(isop, opsc[r, ch], 0)
                scol = strm * 2 * PTs + (2 * j + h) * Q + ul
                mcol = strm * 2 * PTs + h * PTs + j * Q + ul
                sel[core, rowi, scol] = 1.0
                for o in range(NO):
                    pick = ovar == o
                    m30[core[pick], o * NI : (o + 1) * NI, mcol[pick]] = 1.0
            sel_blocks.append(sel)
            m30_blocks.append(m30)
        if p < NP:
            slot_prev = np.full((B, N), -1, np.int64)
            slot_prev[r, n] = ul

    sel_all = np.concatenate(sel_blocks, axis=2)
    m30_all = np.concatenate(m30_blocks, axis=2)

    return dict(
        Qp=Qp, sel=sel_all, m30=m30_all, lr1=lr1,
        r10=r10, gid10=gid[NP], opsc=opsc, litsc=litsc, is_lit=is_lit,
    )


def _build_nc(Qp, SelCols):
    Q10 = int(Qp[NP])
    Qmax = int(max(Qp[1:]))
    PTsmax = GPS * Qmax
    WS = 30 * GPS  # stT / zT free width per stream
    nc = bacc.Bacc(None)
    consts30 = nc.dram_tensor("consts30", [NCONST_ROWS, WS], BF, kind="ExternalInput")
    wmat30 = nc.dram_tensor("wmat30", [100, 30], BF, kind="ExternalInput")
    replpair = nc.dram_tensor("replpair", [30, 200], BF, kind="ExternalInput")
    sel_in = nc.dram_tensor("sel", [128, SelCols], BF, kind="ExternalInput")
    m30_in = nc.dram_tensor("m30", [30, SelCols], BF, kind="ExternalInput")
    PTs1 = GPS * int(Qp[1])
    lr1_in = nc.dram_tensor("lr1", [30, NS * 2 * PTs1], BF, kind="ExternalInput")
    outz = nc.dram_tensor("outz", [Q10, NS * WS], F32, kind="ExternalOutput")

    with ExitStack() as ctx:
        tc = ctx.enter_context(tile.TileContext(nc))
        singles = ctx.enter_context(tc.tile_pool(name="singles", bufs=1))
        psum = ctx.enter_context(tc.tile_pool(name="psum", bufs=1, space="PSUM"))

        # constants -- pass-1 critical ones first, split across both DGE rings
        replp_sb = singles.tile([30, 200], BF)
        nc.scalar.dma_start(out=replp_sb[:, :], in_=replpair[:, :])
        repl_sb = replp_sb[:, 0:100]
        reprm_sb = replp_sb[:, 100:200]
        lr1_sb = singles.tile([30, NS * 2 * PTs1], BF)
        nc.sync.dma_start(out=lr1_sb[:, :], in_=lr1_in[:, :])
        w_sb = singles.tile([100, 30], BF)
        nc.sync.dma_start(out=w_sb[:, :], in_=wmat30[:, :])
        stT = []
        for s in range(NS):
            t = singles.tile([128, WS], BF, tag=f"stT{s}", name=f"stT{s}")
            nc.vector.memset(t[:, :], 0.0)
            nc.scalar.dma_start(out=t[CROW : CROW + NCONST_ROWS, :], in_=consts30[:, :])
            stT.append(t)
        sel_sb = singles.tile([128, SelCols], BF)
        nc.sync.dma_start(out=sel_sb[:, :], in_=sel_in[:, :])
        m30_sb = singles.tile([30, SelCols], BF)
        nc.scalar.dma_start(out=m30_sb[:, :], in_=m30_in[:, :])

        # per-stream work tiles (max size, sliced per pass)
        lr30m = [singles.tile([30, 2 * PTsmax], BF, tag=f"lr30m{s}", name=f"lr30m{s}") for s in range(NS)]
        lsb = [singles.tile([100, PTsmax], F32, tag=f"lsb{s}", name=f"lsb{s}") for s in range(NS)]
        outer = [singles.tile([100, PTsmax], BF, tag=f"outer{s}", name=f"outer{s}") for s in range(NS)]
        esb = [singles.tile([Qmax, WS], F32, tag=f"esb{s}", name=f"esb{s}") for s in range(NS)]
        zs = [singles.tile([Qmax, NO * GPS], F32, tag=f"zs{s}", name=f"zs{s}") for s in range(NS)]
        rz = [singles.tile([Qmax, NO * GPS], F32, tag=f"rz{s}", name=f"rz{s}") for s in range(NS)]
        zout = [singles.tile([Q10, WS], F32, tag=f"zout{s}", name=f"zout{s}") for s in range(NS)]
        lr30p = [psum.tile([30, 2 * PTsmax], F32, tag=f"lr30p{s}", name=f"lr30p{s}") for s in range(NS)]
        ps_l = [psum.tile([100, PTsmax], F32, tag=f"ps_l{s}", name=f"ps_l{s}") for s in range(NS)]
        ps_r = [psum.tile([100, PTsmax], F32, tag=f"ps_r{s}", name=f"ps_r{s}") for s in range(NS)]
        zT = [psum.tile([128, WS], F32, tag=f"zT{s}", name=f"zT{s}") for s in range(NS)]

        soff = 0
        for p in range(1, NP + 1):
            Q = int(Qp[p])
            PTs = GPS * Q
            if p == 1:
                lrm = [lr1_sb[:, s * 2 * PTs : (s + 1) * 2 * PTs] for s in range(NS)]
            else:
                lrm = [lr30m[s][:, 0 : 2 * PTs] for s in range(NS)]
                # selection matmuls + mask-muls, interleaved A/B
                for j in range(GPS):
                    for s in range(NS):
                        off = soff + s * 2 * PTs
                        pslice = lr30p[s][:, 2 * j * Q : 2 * (j + 1) * Q]
                        nc.tensor.matmul(
                            pslice,
                            stT[s][:, 30 * j : 30 * (j + 1)],
                            sel_sb[:, off + 2 * j * Q : off + 2 * (j + 1) * Q],
                            start=True, stop=True,
                        )
                for j in range(GPS):
                    for s in range(NS):
                        off = soff + s * 2 * PTs
                        pslice = lr30p[s][:, 2 * j * Q : 2 * (j + 1) * Q]
                        dst = lrm[s].rearrange("m (h jj u) -> m h jj u",
                                               h=2, jj=GPS)[:, :, j, :]
                        msrc = m30_sb[:, off : off + 2 * PTs].rearrange(
                            "m (h jj u) -> m h jj u", h=2, jj=GPS)[:, :, j, :]
                        nc.vector.tensor_mul(
                            dst, pslice.rearrange("m (h u) -> m h u", h=2), msrc)
                soff += NS * 2 * PTs
            for s in range(NS):
                nc.tensor.matmul(ps_l[s][:, 0:PTs], repl_sb, lrm[s][:, 0:PTs],
                                 start=True, stop=True)
            for s in range(NS):
                nc.scalar.copy(lsb[s][:, 0:PTs], ps_l[s][:, 0:PTs])
                nc.tensor.matmul(ps_r[s][:, 0:PTs], reprm_sb,
                                 lrm[s][:, PTs : 2 * PTs], start=True, stop=True)
            for s in range(NS):
                nc.vector.tensor_mul(outer[s][:, 0:PTs], lsb[s][:, 0:PTs],
                                     ps_r[s][:, 0:PTs])
            for j in range(GPS):
                for s in range(NS):
                    nc.tensor.matmul(
                        zT[s][0:Q, 30 * j : 30 * (j + 1)],
                        outer[s][:, j * Q : (j + 1) * Q],
                        w_sb[:, :],
                        start=True, stop=True,
                    )
            if p == NP:
                for s in range(NS):
                    nc.scalar.copy(zout[s][:, :], zT[s][0:Q, :])
                    nc.sync.dma_start(out=outz[:, s * WS : (s + 1) * WS],
                                      in_=zout[s][:, :])
                continue
            for s in range(NS):
                nc.scalar.activation(esb[s][0:Q, :], zT[s][0:Q, :],
                                     mybir.ActivationFunctionType.Exp)
            for s in range(NS):
                nc.vector.tensor_reduce(
                    zs[s][0:Q, :],
                    esb[s][0:Q, :].rearrange("q (t k) -> q t k", k=NI),
                    axis=mybir.AxisListType.X, op=mybir.AluOpType.add,
                )
            for s in range(NS):
                nc.vector.reciprocal_approx_fast(out=rz[s][0:Q, :], in_=zs[s][0:Q, :])
            for s in range(NS):
                nc.vector.tensor_mul(
                    stT[s][0:Q, :].rearrange("q (t k) -> q t k", k=NI),
                    esb[s][0:Q, :].rearrange("q (t k) -> q t k", k=NI),
                    rz[s][0:Q, :].rearrange("q (t o) -> q t o", o=1).broadcast_to(
                        [Q, NO * GPS, NI]),
                )
    nc.finalize()
    return nc


def kernel(op_table, cats, ops, lits, left, right, mask):
    global LAST_RESULTS
    op_table = np.asarray(op_table, np.float32)
    plan = _plan(np.asarray(cats), np.asarray(ops), np.asarray(lits),
                 np.asarray(left), np.asarray(right), np.asarray(mask))
    Qp = plan["Qp"]
    SelCols = plan["sel"].shape[2]

    nc = _build_nc(Qp, SelCols)

    eyeext = np.concatenate([np.eye(NI, dtype=np.float32),
                             np.zeros((NI, 1), np.float32)], axis=1)
    consts30 = np.zeros((NCONST_ROWS, 30 * GPS), np.float32)
    for s in range(NCONST_ROWS):
        consts30[s, :] = np.tile(eyeext[:, s], NO * GPS)
    w30 = op_table.transpose(1, 2, 0, 3).reshape(100, 30)
    repl = np.kron(np.eye(NI), np.ones((1, NI))).astype(np.float32)
    reprm = np.tile(np.eye(NI), (1, NI)).astype(np.float32)
    repl30 = np.tile(repl, (3, 1))
    reprm30 = np.tile(reprm, (3, 1))

    bf = ml_dtypes.bfloat16
    in_maps = []
    for c in range(NCORES):
        in_maps.append({
            "consts30": consts30.astype(bf),
            "wmat30": w30.astype(bf),
            "replpair": np.concatenate([repl30, reprm30], axis=1).astype(bf),
            "sel": np.ascontiguousarray(plan["sel"][c].astype(bf)),
            "m30": np.ascontiguousarray(plan["m30"][c].astype(bf)),
            "lr1": np.ascontiguousarray(plan["lr1"][c].astype(bf)),
        })

    res = run_bass_kernel_spmd(nc, in_maps, list(range(NCORES)), trace=TRACE)
    LAST_RESULTS = res

    # Assemble the full (B, NI) output on the host (index selection only).
    out = np.zeros((B, NI), np.float32)
    litsc, is_lit = plan["litsc"], plan["is_lit"]
    lit_rows = np.nonzero(cats[:, 0] == 0)[0]
    lr_active = is_lit[lit_rows, 0]
    oh = 10.0 * np.eye(NI, dtype=np.float32)[litsc[lit_rows, 0]]
    out[lit_rows] = np.where(lr_active[:, None], oh, 0.0)

    r10, opsc = plan["r10"], plan["opsc"]
    core10, grp10, ul10 = plan["gid10"]
    WS = 30 * GPS
    for c in range(NCORES):
        z = np.asarray(res.results[c]["outz"])  # (Q10, NS*WS)
        selmask = core10 == c
        rows = r10[selmask]
        u = ul10[selmask]
        g = grp10[selmask]
        o = opsc[rows, 0]
        col0 = (g // GPS) * WS + 30 * (g % GPS) + NI * o
        for k in range(NI):
            out[rows, k] = z[u, col0 + k]
    return out
